# revision 17
# baseline (speedup 1.0000x reference)
"""Chunked causal attention (B=2, nh=16, Tq=1024, Tk=8192, dh=64) on 8 trn2 cores.

Strategy: shard (B*nh)=32 heads -> 4 heads/core, no cross-core comm.

Host-side prep builds the exact SBUF images the PE wants, so the device does
zero layout work: K^T arrives row-paired ([d, k] with even k-tiles on
partitions 0-63, odd on 64-127), Q^T arrives duplicated across both halves,
and V arrives tile-major with a ones column appended (so the PV accumulator
row 64 is the softmax denominator).

Per head: S^T = K @ Q^T / 8 in [k_tile=128 part, q free] layout, PE
row-paired via tile_position (contraction d=64 fills half the array; two
k-tiles run concurrently on row groups 0-1 / 2-3). exp on ScalarE straight
out of PSUM in [128, 1536] batches (3 PSUM banks per batch, double-buffered
= 6 banks; scale 1/8 fused), causal mask as a 0/1 multiply on diagonal
groups only, then PV matmul with V|ones stationary. The [65, 512]
accumulators go raw to DRAM; divide-by-denominator + transpose happen on
host.

All matmul operands are float32r (1 cycle/row on the PE), fp32 accumulation
in PSUM. ScalarE exp and the PE matmul stream are co-critical (~250us).
"""

import base64
import io

import numpy as np

import concourse.bacc as bacc
import concourse.bass as bass
import concourse.tile as tile
from concourse import mybir
from concourse.bass_utils import run_bass_kernel_spmd

F32 = mybir.dt.float32
F32R = mybir.dt.float32r

N_CORES = 8
B, NH, TQ, TK, D = 2, 16, 1024, 8192, 64
H = (B * NH) // N_CORES          # heads per core = 4
KT_TILES = TK // 128             # 64 k-tiles of 128
KP = KT_TILES // 2               # 32 k-tile pairs
QB = TQ // 512                   # 2 q-blocks of 512
KCH = 16                         # K load chunks per head
VCH = 4                          # V load chunks per head

# k-tile groups per q-block: exp batch = GSIZE tiles = GSIZE PSUM banks.
# PSUM budget: 2 x GSIZE (s double-buffer) + 2 (o_acc double-buffer) = 8.
GSIZE = 3
GROUPS = [(GSIZE * g, GSIZE) for g in range(KT_TILES // GSIZE)]
if KT_TILES % GSIZE:
    GROUPS.append((GSIZE * (KT_TILES // GSIZE), KT_TILES % GSIZE))
GW = GSIZE * 512                 # max group width in columns


def _mask_info(q_chunk_start):
    """Per (group gi, q-block qb): status plus per-tile keep flags.
    status: 'full' | 'skip' | ('mask', idx). Masks padded to [128, GW] with
    ones. tile_keep[(gi, qb)] = list of per-tile 'any kept' bools."""
    info = {}
    tile_keep = {}
    masks = []
    for gi, (g0, ng) in enumerate(GROUPS):
        for qb in range(QB):
            qg = q_chunk_start + 512 * qb + np.arange(512)
            keeps = []
            for i in range(ng):
                kg = 128 * (g0 + i) + np.arange(128)
                keeps.append(qg[None, :] >= kg[:, None])     # [128, 512]
            cat = np.concatenate(keeps, axis=1)
            tile_keep[(gi, qb)] = [k.any() for k in keeps]
            if cat.all():
                info[(gi, qb)] = ("full", None)
            elif not cat.any():
                info[(gi, qb)] = ("skip", None)
            else:
                pad = np.ones((128, GW), dtype=np.float32)
                pad[:, :cat.shape[1]] = cat
                info[(gi, qb)] = ("mask", len(masks))
                masks.append(pad)
    mask_arr = (np.stack(masks) if masks
                else np.zeros((1, 128, GW), dtype=np.float32))
    return info, tile_keep, mask_arr


def _inline_f32r(nc, data, name):
    """inline_tensor, but declared float32r (same bits as float32) so plain
    HWDGE DMAs into float32r SBUF tiles need no gpsimd cast."""
    data = np.ascontiguousarray(data.astype(np.float32))
    mls = nc._tensor(name, list(data.shape), F32R, kind="Const", type="DRAM")
    buf = io.BytesIO()
    np.save(buf, data, allow_pickle=False)
    mls.file = f"{name}.npy"
    mls.ant_data = base64.standard_b64encode(buf.getvalue()).decode()
    return bass.DRamTensorHandle(name, list(data.shape), F32R)


def build_nc(q_chunk_start):
    nc = bacc.Bacc("TRN2", target_bir_lowering=False, debug=False)

    # host ships PE-ready layouts; declaring DRAM float32r is a pure relabel.
    kt_d = nc.dram_tensor("kt", [H, 128, KP * 128], F32R, kind="ExternalInput")
    qt_d = nc.dram_tensor("qt", [H, 128, TQ], F32R, kind="ExternalInput")
    vx_d = nc.dram_tensor("vx", [H, 128, KT_TILES, D + 1], F32R,
                          kind="ExternalInput")
    o_d = nc.dram_tensor("o", [H, QB, D + 1, 512], F32, kind="ExternalOutput")

    info, tile_keep, mask_arr = _mask_info(q_chunk_start)
    n_masks = mask_arr.shape[0]
    masks_d = _inline_f32r(nc, mask_arr, "cmasks")

    with tile.TileContext(nc) as tc:
        with (
            tc.tile_pool(name="const", bufs=1) as const,
            tc.tile_pool(name="ktp", bufs=2) as ktp,
            tc.tile_pool(name="qtp", bufs=2) as qtp,
            tc.tile_pool(name="vp", bufs=2) as vp,
            tc.tile_pool(name="xp", bufs=4) as xp,
            tc.tile_pool(name="ostage", bufs=2) as ostage,
            tc.tile_pool(name="s_ps", bufs=2, space="PSUM") as s_ps,
            tc.tile_pool(name="o_ps", bufs=2, space="PSUM") as o_ps,
        ):
            mask_sb = const.tile([128, n_masks, GW], F32R)

            def load_masks():
                nc.sync.dma_start(
                    mask_sb[:], masks_d.ap().rearrange("m p f -> p m f"))

            # per-head persistent tiles
            kt = {}    # h -> [128, KP*128] row-paired K^T
            qt = {}    # h -> [128, TQ] duplicated Q^T
            vsb = {}   # h -> [128, KT_TILES, D+1] V | ones

            def emit_loads(h):
                kt[h] = ktp.tile([128, KP * 128], F32R, tag="kt", name=f"kt{h}")
                qt[h] = qtp.tile([128, TQ], F32R, tag="qt", name=f"qt{h}")
                vsb[h] = vp.tile([128, KT_TILES, D + 1], F32R, tag="v",
                                 name=f"v{h}")
                # qt + first kt chunk + first vx chunk lead so head 0's
                # first S-matmuls and first PV batch aren't stuck behind
                # the rest of the K/V traffic at boot; then K/V interleave
                # in consumption order.
                def kc(c):
                    cw = (KP * 128) // KCH
                    nc.sync.dma_start(
                        kt[h][:, c * cw:(c + 1) * cw],
                        kt_d[h][:, c * cw:(c + 1) * cw])

                def vc(c):
                    tpv = KT_TILES // VCH
                    nc.sync.dma_start(
                        vsb[h][:, c * tpv:(c + 1) * tpv, :],
                        vx_d[h][:, c * tpv:(c + 1) * tpv, :])

                nc.sync.dma_start(qt[h][:, 0:512], qt_d[h][:, 0:512])
                kc(0)
                kc(1)
                nc.sync.dma_start(qt[h][:, 512:TQ], qt_d[h][:, 512:TQ])
                vc(0)
                kc(2)
                kc(3)
                kc(4)
                kc(5)
                vc(1)
                kc(6)
                kc(7)
                kc(8)
                kc(9)
                vc(2)
                kc(10)
                kc(11)
                kc(12)
                kc(13)
                vc(3)
                kc(14)
                kc(15)

            def release(h):
                del kt[h], qt[h], vsb[h]

            # deferred tail of the previous (head, qb): its last PV batches +
            # output DMA are emitted one-per-group after the NEXT block's
            # first S-matmuls so ScalarE never waits on the PV backlog at
            # block boundaries.
            pending_steps = []

            def main(h, qb):
                active = [gi for gi in range(len(GROUPS))
                          if info[(gi, qb)][0] != "skip"]
                n_pv = sum(sum(tile_keep[(gi, qb)]) for gi in active)
                o_acc = o_ps.tile([D + 1, 512], F32, tag="oacc")
                pv_state = {"i": 0}
                v_tile = vsb[h]   # bind now: the deferred tail outlives release(h)

                def emit_pv(gi, x_t):
                    g0, ng = GROUPS[gi]
                    keep = tile_keep[(gi, qb)]
                    for i in range(ng):
                        if not keep[i]:
                            continue
                        nc.tensor.matmul(
                            o_acc[:],
                            v_tile[:, g0 + i, :],
                            x_t[:, 512 * i:512 * (i + 1)],
                            start=(pv_state["i"] == 0),
                            stop=(pv_state["i"] == n_pv - 1),
                        )
                        pv_state["i"] += 1

                def emit_s(gi):
                    g0, ng = GROUPS[gi]
                    keep = tile_keep[(gi, qb)]
                    s_t = s_ps.tile([128, GW], F32, tag="s")
                    for i in range(ng):
                        if not keep[i]:
                            continue
                        t = g0 + i
                        rb = 64 * (t % 2)
                        nc.tensor.matmul(
                            s_t[:, 512 * i:512 * (i + 1)],
                            kt[h][rb:rb + 64, 128 * (t // 2):128 * (t // 2 + 1)],
                            qt[h][rb:rb + 64, 512 * qb:512 * (qb + 1)],
                            start=True, stop=True, tile_position=(rb, 0),
                        )
                    return s_t

                is_last = (h == H - 1 and qb == QB - 1)
                # PV for group g normally trails act(g+1); at block
                # boundaries the last 1-2 PV batches are deferred into the
                # next block (after its first S-matmuls) so ScalarE never
                # waits on the PV backlog. The final block emits eagerly.
                n_defer = 1 if is_last else 2
                prev = None
                deferred = []
                for pos, gi in enumerate(active):
                    kind, mask_i = info[(gi, qb)]
                    w = 512 * GROUPS[gi][1]
                    s_t = emit_s(gi)
                    if pending_steps:
                        pending_steps.pop(0)()
                    x_t = xp.tile([128, GW], F32R, tag="x")
                    nc.scalar.activation(
                        x_t[:, 0:w], s_t[:, 0:w],
                        mybir.ActivationFunctionType.Exp,
                        scale=1.0 / np.sqrt(D),
                    )
                    if kind == "mask":
                        nc.vector.tensor_mul(
                            x_t[:, 0:w], x_t[:, 0:w], mask_sb[:, mask_i, 0:w])
                    if prev is not None:
                        if pos >= len(active) - (n_defer - 1):
                            deferred.append(prev)
                        else:
                            emit_pv(*prev)
                    prev = (gi, x_t)
                deferred.append(prev)

                def emit_out(h=h, qb=qb, o_acc=o_acc):
                    # raw [65, 512] accumulator to DRAM (PSUM can't DMA
                    # directly); divide-by-denominator + transpose on host.
                    osb = ostage.tile([D + 1, 512], F32, tag="osb")
                    nc.vector.tensor_copy(osb[:], o_acc[:])
                    nc.sync.dma_start(o_d[h, qb], osb[:])

                for pv in deferred[:-1]:
                    pending_steps.append(
                        lambda pv=pv, emit_pv=emit_pv: emit_pv(*pv))

                def last_step(pv=deferred[-1], emit_pv=emit_pv,
                              emit_out=emit_out):
                    emit_pv(*pv)
                    emit_out()
                pending_steps.append(last_step)

            emit_loads(0)
            load_masks()
            for h in range(H):
                if h + 1 < H:
                    emit_loads(h + 1)
                for qb in range(QB):
                    main(h, qb)
                release(h)
            while pending_steps:
                pending_steps.pop(0)()
    nc.compile()
    return nc


_CACHE = {}


def _get_nc(q_chunk_start):
    key = int(q_chunk_start)
    if key not in _CACHE:
        _CACHE[key] = build_nc(key)
    return _CACHE[key]


def _prep_inputs(q, k, v):
    """Build per-head PE-ready layouts on host (cheap numpy strided copies).

    kt: [AH, 128, KP*128]  row-paired K^T: partition 64*(t%2)+d,
        col 128*(t//2)+j  holds K[h, 128*t+j, d]
    qt: [AH, 128, TQ]      Q^T duplicated on partitions 0-63 / 64-127
    vx: [AH, 128, KT, 65]  vx[h, p, t, d] = V[h, 128*t+p, d]; [..., 64] = 1
    """
    ah = B * NH
    k4 = k.reshape(ah, KP, 2, 128, D)            # h, pair, parity, k, d
    kt = np.ascontiguousarray(
        k4.transpose(0, 2, 4, 1, 3).reshape(ah, 128, KP * 128))
    qT = q.transpose(0, 2, 1)                    # [ah, D, TQ]
    qt = np.ascontiguousarray(
        np.concatenate([qT, qT], axis=1))        # [ah, 128, TQ]
    v4 = v.reshape(ah, KT_TILES, 128, D).transpose(0, 2, 1, 3)
    vx = np.empty((ah, 128, KT_TILES, D + 1), dtype=np.float32)
    vx[..., :D] = v4
    vx[..., D] = 1.0
    return kt, qt, vx


def kernel(q, k, v, q_chunk_start, _trace=False):
    q = np.ascontiguousarray(np.asarray(q, dtype=np.float32)).reshape(B * NH, TQ, D)
    k = np.ascontiguousarray(np.asarray(k, dtype=np.float32)).reshape(B * NH, TK, D)
    v = np.ascontiguousarray(np.asarray(v, dtype=np.float32)).reshape(B * NH, TK, D)
    qcs = int(np.asarray(q_chunk_start))

    kt, qt, vx = _prep_inputs(q, k, v)
    nc = _get_nc(qcs)
    in_maps = []
    for c in range(N_CORES):
        s = slice(c * H, (c + 1) * H)
        in_maps.append({
            "kt": np.ascontiguousarray(kt[s]),
            "qt": np.ascontiguousarray(qt[s]),
            "vx": np.ascontiguousarray(vx[s]),
        })
    res = run_bass_kernel_spmd(
        nc, in_maps, core_ids=list(range(N_CORES)), trace=_trace)
    raw = np.stack([res.results[c]["o"] for c in range(N_CORES)])
    # raw: [cores, H, QB, 65, 512]; row 64 is the softmax denominator
    num = raw[:, :, :, 0:D, :]
    den = raw[:, :, :, D:D + 1, :]
    out = (num / den).transpose(0, 1, 2, 4, 3)          # [c, H, QB, 512, D]
    out = out.reshape(B, NH, TQ, D)
    if _trace:
        kernel._last_exec_time_ns = res.exec_time_ns
        kernel._last_results = res
    return out


# revision 18
# speedup vs baseline: 1.0112x; 1.0112x over previous
"""Chunked causal attention (B=2, nh=16, Tq=1024, Tk=8192, dh=64) on 8 trn2 cores.

Strategy: shard (B*nh)=32 heads -> 4 heads/core, no cross-core comm.

Host-side prep builds the exact SBUF images the PE wants, so the device does
zero layout work: K^T arrives row-paired ([d, k] with even k-tiles on
partitions 0-63, odd on 64-127), Q^T arrives duplicated across both halves,
and V arrives tile-major with a ones column appended (so the PV accumulator
row 64 is the softmax denominator).

Per head: S^T = K @ Q^T / 8 in [k_tile=128 part, q free] layout, PE
row-paired via tile_position (contraction d=64 fills half the array; two
k-tiles run concurrently on row groups 0-1 / 2-3). exp on ScalarE straight
out of PSUM in [128, 1536] batches (3 PSUM banks per batch, double-buffered
= 6 banks; scale 1/8 fused), causal mask as a 0/1 multiply on diagonal
groups only, then PV matmul with V|ones stationary. The [65, 512]
accumulators go raw to DRAM; divide-by-denominator + transpose happen on
host.

All matmul operands are float32r (1 cycle/row on the PE), fp32 accumulation
in PSUM. ScalarE exp and the PE matmul stream are co-critical (~250us).
"""

import base64
import io

import numpy as np

import concourse.bacc as bacc
import concourse.bass as bass
import concourse.tile as tile
from concourse import mybir
from concourse.bass_utils import run_bass_kernel_spmd

F32 = mybir.dt.float32
F32R = mybir.dt.float32r

N_CORES = 8
B, NH, TQ, TK, D = 2, 16, 1024, 8192, 64
H = (B * NH) // N_CORES          # heads per core = 4
KT_TILES = TK // 128             # 64 k-tiles of 128
KP = KT_TILES // 2               # 32 k-tile pairs
QB = TQ // 512                   # 2 q-blocks of 512
KCH = 16                         # K load chunks per head
VCH = 4                          # V load chunks per head

# k-tile groups per q-block: exp batch = GSIZE tiles = GSIZE PSUM banks.
# PSUM budget: 2 x GSIZE (s double-buffer) + 2 (o_acc double-buffer) = 8.
GSIZE = 3
GROUPS = [(GSIZE * g, GSIZE) for g in range(KT_TILES // GSIZE)]
if KT_TILES % GSIZE:
    GROUPS.append((GSIZE * (KT_TILES // GSIZE), KT_TILES % GSIZE))
GW = GSIZE * 512                 # max group width in columns


def _mask_info(q_chunk_start):
    """Per (group gi, q-block qb): status plus per-tile keep flags.
    status: 'full' | 'skip' | ('mask', idx). Masks padded to [128, GW] with
    ones. tile_keep[(gi, qb)] = list of per-tile 'any kept' bools."""
    info = {}
    tile_keep = {}
    masks = []
    for gi, (g0, ng) in enumerate(GROUPS):
        for qb in range(QB):
            qg = q_chunk_start + 512 * qb + np.arange(512)
            keeps = []
            for i in range(ng):
                kg = 128 * (g0 + i) + np.arange(128)
                keeps.append(qg[None, :] >= kg[:, None])     # [128, 512]
            cat = np.concatenate(keeps, axis=1)
            tile_keep[(gi, qb)] = [k.any() for k in keeps]
            if cat.all():
                info[(gi, qb)] = ("full", None)
            elif not cat.any():
                info[(gi, qb)] = ("skip", None)
            else:
                pad = np.ones((128, GW), dtype=np.float32)
                pad[:, :cat.shape[1]] = cat
                info[(gi, qb)] = ("mask", len(masks))
                masks.append(pad)
    mask_arr = (np.stack(masks) if masks
                else np.zeros((1, 128, GW), dtype=np.float32))
    return info, tile_keep, mask_arr


def _inline_f32r(nc, data, name):
    """inline_tensor, but declared float32r (same bits as float32) so plain
    HWDGE DMAs into float32r SBUF tiles need no gpsimd cast."""
    data = np.ascontiguousarray(data.astype(np.float32))
    mls = nc._tensor(name, list(data.shape), F32R, kind="Const", type="DRAM")
    buf = io.BytesIO()
    np.save(buf, data, allow_pickle=False)
    mls.file = f"{name}.npy"
    mls.ant_data = base64.standard_b64encode(buf.getvalue()).decode()
    return bass.DRamTensorHandle(name, list(data.shape), F32R)


def build_nc(q_chunk_start):
    nc = bacc.Bacc("TRN2", target_bir_lowering=False, debug=False)

    # host ships PE-ready layouts; declaring DRAM float32r is a pure relabel.
    kt_d = nc.dram_tensor("kt", [H, 128, KP * 128], F32R, kind="ExternalInput")
    qt_d = nc.dram_tensor("qt", [H, 128, TQ], F32R, kind="ExternalInput")
    vx_d = nc.dram_tensor("vx", [H, 128, KT_TILES, D + 1], F32R,
                          kind="ExternalInput")
    o_d = nc.dram_tensor("o", [H, QB, D + 1, 512], F32, kind="ExternalOutput")

    info, tile_keep, mask_arr = _mask_info(q_chunk_start)
    n_masks = mask_arr.shape[0]
    masks_d = _inline_f32r(nc, mask_arr, "cmasks")

    with tile.TileContext(nc) as tc:
        with (
            tc.tile_pool(name="const", bufs=1) as const,
            tc.tile_pool(name="ktp", bufs=2) as ktp,
            tc.tile_pool(name="qtp", bufs=2) as qtp,
            tc.tile_pool(name="vp", bufs=2) as vp,
            tc.tile_pool(name="xp", bufs=4) as xp,
            tc.tile_pool(name="ostage", bufs=2) as ostage,
            tc.tile_pool(name="s_ps", bufs=2, space="PSUM") as s_ps,
            tc.tile_pool(name="o_ps", bufs=2, space="PSUM") as o_ps,
        ):
            mask_sb = const.tile([128, n_masks, GW], F32R)

            def load_masks():
                nc.sync.dma_start(
                    mask_sb[:], masks_d.ap().rearrange("m p f -> p m f"))

            # per-head persistent tiles
            kt = {}    # h -> [128, KP*128] row-paired K^T
            qt = {}    # h -> [128, TQ] duplicated Q^T
            vsb = {}   # h -> [128, KT_TILES, D+1] V | ones

            def emit_loads(h):
                kt[h] = ktp.tile([128, KP * 128], F32R, tag="kt", name=f"kt{h}")
                qt[h] = qtp.tile([128, TQ], F32R, tag="qt", name=f"qt{h}")
                vsb[h] = vp.tile([128, KT_TILES, D + 1], F32R, tag="v",
                                 name=f"v{h}")
                # qt + first kt chunk + first vx chunk lead so head 0's
                # first S-matmuls and first PV batch aren't stuck behind
                # the rest of the K/V traffic at boot; then K/V interleave
                # in consumption order.
                def kc(c):
                    cw = (KP * 128) // KCH
                    nc.sync.dma_start(
                        kt[h][:, c * cw:(c + 1) * cw],
                        kt_d[h][:, c * cw:(c + 1) * cw])

                def vc(c):
                    tpv = KT_TILES // VCH
                    nc.sync.dma_start(
                        vsb[h][:, c * tpv:(c + 1) * tpv, :],
                        vx_d[h][:, c * tpv:(c + 1) * tpv, :])

                nc.sync.dma_start(qt[h][:, 0:512], qt_d[h][:, 0:512])
                kc(0)
                kc(1)
                nc.sync.dma_start(qt[h][:, 512:TQ], qt_d[h][:, 512:TQ])
                vc(0)
                kc(2)
                kc(3)
                kc(4)
                kc(5)
                vc(1)
                kc(6)
                kc(7)
                kc(8)
                kc(9)
                vc(2)
                kc(10)
                kc(11)
                kc(12)
                kc(13)
                vc(3)
                kc(14)
                kc(15)

            def release(h):
                del kt[h], qt[h], vsb[h]

            # deferred tail of the previous (head, qb): its last PV batches +
            # output DMA are emitted one-per-group after the NEXT block's
            # first S-matmuls so ScalarE never waits on the PV backlog at
            # block boundaries.
            pending_steps = []

            def main(h, qb):
                active = [gi for gi in range(len(GROUPS))
                          if info[(gi, qb)][0] != "skip"]
                n_pv = sum(sum(tile_keep[(gi, qb)]) for gi in active)
                o_acc = o_ps.tile([D + 1, 512], F32, tag="oacc")
                pv_state = {"i": 0}
                v_tile = vsb[h]   # bind now: the deferred tail outlives release(h)

                def emit_pv(gi, x_t):
                    g0, ng = GROUPS[gi]
                    keep = tile_keep[(gi, qb)]
                    for i in range(ng):
                        if not keep[i]:
                            continue
                        nc.tensor.matmul(
                            o_acc[:],
                            v_tile[:, g0 + i, :],
                            x_t[:, 512 * i:512 * (i + 1)],
                            start=(pv_state["i"] == 0),
                            stop=(pv_state["i"] == n_pv - 1),
                        )
                        pv_state["i"] += 1

                def emit_s(gi):
                    g0, ng = GROUPS[gi]
                    keep = tile_keep[(gi, qb)]
                    s_t = s_ps.tile([128, GW], F32, tag="s")
                    for i in range(ng):
                        if not keep[i]:
                            continue
                        t = g0 + i
                        rb = 64 * (t % 2)
                        nc.tensor.matmul(
                            s_t[:, 512 * i:512 * (i + 1)],
                            kt[h][rb:rb + 64, 128 * (t // 2):128 * (t // 2 + 1)],
                            qt[h][rb:rb + 64, 512 * qb:512 * (qb + 1)],
                            start=True, stop=True, tile_position=(rb, 0),
                        )
                    return s_t

                is_last = (h == H - 1 and qb == QB - 1)
                if is_last:
                    # masked groups first: keeps the mask-multiply latency
                    # (act -> DVE mask -> PV) off the kernel's final tail.
                    active = ([gi for gi in active if info[(gi, qb)][0] == "mask"]
                              + [gi for gi in active if info[(gi, qb)][0] != "mask"])
                # PV for group g normally trails act(g+1); at block
                # boundaries the last 1-2 PV batches are deferred into the
                # next block (after its first S-matmuls) so ScalarE never
                # waits on the PV backlog. The final block emits eagerly.
                n_defer = 1 if is_last else 2
                prev = None
                deferred = []
                for pos, gi in enumerate(active):
                    kind, mask_i = info[(gi, qb)]
                    w = 512 * GROUPS[gi][1]
                    s_t = emit_s(gi)
                    if pending_steps:
                        pending_steps.pop(0)()
                    x_t = xp.tile([128, GW], F32R, tag="x")
                    nc.scalar.activation(
                        x_t[:, 0:w], s_t[:, 0:w],
                        mybir.ActivationFunctionType.Exp,
                        scale=1.0 / np.sqrt(D),
                    )
                    if kind == "mask":
                        nc.vector.tensor_mul(
                            x_t[:, 0:w], x_t[:, 0:w], mask_sb[:, mask_i, 0:w])
                    if prev is not None:
                        if pos >= len(active) - (n_defer - 1):
                            deferred.append(prev)
                        else:
                            emit_pv(*prev)
                    prev = (gi, x_t)
                deferred.append(prev)

                def emit_out(h=h, qb=qb, o_acc=o_acc):
                    # raw [65, 512] accumulator to DRAM (PSUM can't DMA
                    # directly); divide-by-denominator + transpose on host.
                    osb = ostage.tile([D + 1, 512], F32, tag="osb")
                    nc.vector.tensor_copy(osb[:], o_acc[:])
                    nc.sync.dma_start(o_d[h, qb], osb[:])

                for pv in deferred[:-1]:
                    pending_steps.append(
                        lambda pv=pv, emit_pv=emit_pv: emit_pv(*pv))

                def last_step(pv=deferred[-1], emit_pv=emit_pv,
                              emit_out=emit_out):
                    emit_pv(*pv)
                    emit_out()
                pending_steps.append(last_step)

            emit_loads(0)
            load_masks()
            for h in range(H):
                if h + 1 < H:
                    emit_loads(h + 1)
                for qb in range(QB):
                    main(h, qb)
                release(h)
            while pending_steps:
                pending_steps.pop(0)()
    nc.compile()
    return nc


_CACHE = {}


def _get_nc(q_chunk_start):
    key = int(q_chunk_start)
    if key not in _CACHE:
        _CACHE[key] = build_nc(key)
    return _CACHE[key]


def _prep_inputs(q, k, v):
    """Build per-head PE-ready layouts on host (cheap numpy strided copies).

    kt: [AH, 128, KP*128]  row-paired K^T: partition 64*(t%2)+d,
        col 128*(t//2)+j  holds K[h, 128*t+j, d]
    qt: [AH, 128, TQ]      Q^T duplicated on partitions 0-63 / 64-127
    vx: [AH, 128, KT, 65]  vx[h, p, t, d] = V[h, 128*t+p, d]; [..., 64] = 1
    """
    ah = B * NH
    k4 = k.reshape(ah, KP, 2, 128, D)            # h, pair, parity, k, d
    kt = np.ascontiguousarray(
        k4.transpose(0, 2, 4, 1, 3).reshape(ah, 128, KP * 128))
    qT = q.transpose(0, 2, 1)                    # [ah, D, TQ]
    qt = np.ascontiguousarray(
        np.concatenate([qT, qT], axis=1))        # [ah, 128, TQ]
    v4 = v.reshape(ah, KT_TILES, 128, D).transpose(0, 2, 1, 3)
    vx = np.empty((ah, 128, KT_TILES, D + 1), dtype=np.float32)
    vx[..., :D] = v4
    vx[..., D] = 1.0
    return kt, qt, vx


def kernel(q, k, v, q_chunk_start, _trace=False):
    q = np.ascontiguousarray(np.asarray(q, dtype=np.float32)).reshape(B * NH, TQ, D)
    k = np.ascontiguousarray(np.asarray(k, dtype=np.float32)).reshape(B * NH, TK, D)
    v = np.ascontiguousarray(np.asarray(v, dtype=np.float32)).reshape(B * NH, TK, D)
    qcs = int(np.asarray(q_chunk_start))

    kt, qt, vx = _prep_inputs(q, k, v)
    nc = _get_nc(qcs)
    in_maps = []
    for c in range(N_CORES):
        s = slice(c * H, (c + 1) * H)
        in_maps.append({
            "kt": np.ascontiguousarray(kt[s]),
            "qt": np.ascontiguousarray(qt[s]),
            "vx": np.ascontiguousarray(vx[s]),
        })
    res = run_bass_kernel_spmd(
        nc, in_maps, core_ids=list(range(N_CORES)), trace=_trace)
    raw = np.stack([res.results[c]["o"] for c in range(N_CORES)])
    # raw: [cores, H, QB, 65, 512]; row 64 is the softmax denominator
    num = raw[:, :, :, 0:D, :]
    den = raw[:, :, :, D:D + 1, :]
    out = (num / den).transpose(0, 1, 2, 4, 3)          # [c, H, QB, 512, D]
    out = out.reshape(B, NH, TQ, D)
    if _trace:
        kernel._last_exec_time_ns = res.exec_time_ns
        kernel._last_results = res
    return out
